# revision 1
# baseline (speedup 1.0000x reference)
"""Trainium2 Bass kernel for nn_CANN_39994735460546.

Reference semantics:
  t    = (physical_params[:, :, None] ** PS_POWERS).reshape(B, 64)
  norm = (t - t.mean()) / t.std(ddof=1)          # global scalar stats
  h    = relu(norm) @ W1.T + b1
  c    = h @ W2.T + b2                            # [B, 5]
  dy[b, j] = sum_k c[b,k] * p_k * eta[b,j]^(p_k - 1),  p = [2,5,8,11,14]

Device strategy (8 NeuronCores, pure data parallel over eta rows; each core
owns 512 rows; stage 1 replicated on every core so no collectives needed):
  - No activation between the two linears -> fuse on host:
      Weff = W2 @ W1, beff = W2 @ b1 + b2, and fold the p_k factors in.
  - Stage 1 (tiny): load params.T [4, B] (rolled per core so its own rows
    come first -> identical SPMD program on all cores), ln on ACT,
    replicate+scale to [64, B] via a PE matmul with a host-built (4 x 64)
    selection matrix holding PS_POWERS, exp on ACT, global stats via DVE
    accum_out + a ones-matmul partition reduction, relu-normalize, one
    matmul to coefficients cT [5, B], PE-transpose 128-col slices to
    [128, 5] per-row-block coefficient tiles.
  - Stage 2 (the heavy part): with u = eta^3,
      dy = ((((c4*u + c3)*u) + c2)*u + c1)*u + c0)*eta
    all on DVE (exact multiplies; one engine -> minimal cross-engine
    synchronization, which is expensive on this platform).
  - DMA: two [128, 2x4096] double-tiles per core pass (16KB contiguous per
    descriptor row), loads on the SP HWDGE queue, stores on the ACT HWDGE
    queue -- the two hardware DMA queues available.
"""

import sys
import numpy as np

sys.path.insert(0, "/opt/trn_rl_repo")

B = 4096
L = 4096
NCORES = 8
RPC = B // NCORES          # rows per core = 512
NPT = RPC // 128           # 128-row blocks per core = 4
CT = 4096                  # row width
NDT = 2                    # double-tiles per pass (each covers 256 rows)
NTOT = float(B * 64)       # elements in t for the global stats

PS_POWERS = np.array([-5.0, -4.0, -3.0, -2.0, -1.5, -1.0, -0.5, 0.0,
                      0.5, 2.0, 1.0 / 3.0, 3.0, 0.25, 4.0, 0.2, 5.0],
                     dtype=np.float32)
POLY_POWERS = np.array([2.0, 5.0, 8.0, 11.0, 14.0], dtype=np.float32)

_cache = {}


def _build_nc(repeat=1):
    import concourse.bass as bass
    import concourse.tile as tile
    from concourse import bacc, mybir

    F32 = mybir.dt.float32
    AF = mybir.ActivationFunctionType
    OP = mybir.AluOpType
    ts = bass.ts

    k1 = 1.0 / (NTOT - 1.0)
    k2 = 1.0 / (NTOT * (NTOT - 1.0))

    nc = bacc.Bacc("TRN2", target_bir_lowering=False, debug=False,
                   num_devices=NCORES)

    eta_d = nc.dram_tensor("eta", [RPC, L], F32, kind="ExternalInput").ap()
    pT_d = nc.dram_tensor("pT", [4, B], F32, kind="ExternalInput").ap()
    rm_d = nc.dram_tensor("rm", [4, 64], F32, kind="ExternalInput").ap()
    wpT_d = nc.dram_tensor("wpT", [64, 5], F32, kind="ExternalInput").ap()
    bp_d = nc.dram_tensor("bp", [5, 1], F32, kind="ExternalInput").ap()
    ones64_d = nc.dram_tensor("ones64", [64, 1], F32, kind="ExternalInput").ap()
    onesr_d = nc.dram_tensor("onesr", [1, 64], F32, kind="ExternalInput").ap()
    eye5_d = nc.dram_tensor("eye5", [5, 5], F32, kind="ExternalInput").ap()
    dy_d = nc.dram_tensor("dy", [RPC, L], F32, kind="ExternalOutput").ap()

    with tile.TileContext(nc) as tc:
        with (
            tc.tile_pool(name="consts", bufs=1) as p_const,
            tc.tile_pool(name="ps_small", bufs=1, space="PSUM") as p_pss,
            tc.tile_pool(name="ps_r", bufs=2, space="PSUM") as p_psr,
            tc.tile_pool(name="ps_c", bufs=2, space="PSUM") as p_psc,
            tc.tile_pool(name="ps_t", bufs=2, space="PSUM") as p_pst,
        ):
            # ---- constants (on the store queue; loads keep the SP queue) --
            rm_sb = p_const.tile([4, 64], F32, tag="rm")
            nc.scalar.dma_start(rm_sb[:], rm_d)
            wpT_sb = p_const.tile([64, 5], F32, tag="wpT")
            nc.scalar.dma_start(wpT_sb[:], wpT_d)
            bp_sb = p_const.tile([5, 1], F32, tag="bp")
            nc.scalar.dma_start(bp_sb[:], bp_d)
            ones64_sb = p_const.tile([64, 1], F32, tag="ones64")
            nc.scalar.dma_start(ones64_sb[:], ones64_d)
            onesr_sb = p_const.tile([1, 64], F32, tag="onesr")
            nc.scalar.dma_start(onesr_sb[:], onesr_d)
            eye5_sb = p_const.tile([5, 5], F32, tag="eye5")
            nc.scalar.dma_start(eye5_sb[:], eye5_d)
            ctiles = [p_const.tile([128, 5], F32, tag=f"ct{t}",
                                   name=f"ct{t}") for t in range(NPT)]

            # ---- stage 1 in its own (stack-freed) scratch pool ----
            with tc.tile_pool(name="s1", bufs=1) as p_s1:
                pT_sb = p_s1.tile([4, B], F32, tag="pT")
                nc.scalar.dma_start(pT_sb[:], pT_d)
                t64 = p_s1.tile([64, B], F32, tag="t64")
                scr64 = p_s1.tile([64, B], F32, tag="scr64")
                s12 = p_s1.tile([64, 2], F32, tag="s12")

                # ln(params.T); t64 = exp(rm.T @ ln)  (rm replicates 4->64
                # rows and scales by PS_POWERS)
                nc.scalar.activation(pT_sb[:], pT_sb[:], AF.Ln)
                for j in range(B // 512):
                    ps_r = p_psr.tile([64, 512], F32, tag="ps_r")
                    nc.tensor.matmul(ps_r[:], rm_sb[:], pT_sb[:, ts(j, 512)],
                                     start=True, stop=True)
                    nc.scalar.activation(t64[:, ts(j, 512)], ps_r[:], AF.Exp)

                # S2 = sum(t^2), S1 = sum(t) per partition (DVE accum_out)
                nc.vector.scalar_tensor_tensor(scr64[:], t64[:], 1.0, t64[:],
                                               OP.mult, OP.mult,
                                               accum_out=s12[:, 1:2])
                nc.vector.tensor_scalar(scr64[:], t64[:], 1.0, 0.0, OP.mult,
                                        OP.add, accum_out=s12[:, 0:1])

                # cross-partition: [1,2] = ones64.T @ s12
                ps_s = p_pss.tile([1, 2], F32, tag="ps_s")
                nc.tensor.matmul(ps_s[:], ones64_sb[:], s12[:],
                                 start=True, stop=True)
                s12sb = p_s1.tile([1, 2], F32, tag="s12sb")
                nc.vector.tensor_copy(s12sb[:], ps_s[:])

                # var = S2/(N-1) - S1^2/(N(N-1)); inv_std = exp(-0.5 ln var)
                scr = p_s1.tile([1, 4], F32, tag="scr")
                ab = p_s1.tile([1, 2], F32, tag="ab")
                nc.vector.tensor_scalar(scr[:, 0:1], s12sb[:, 0:1],
                                        s12sb[:, 0:1], -k2, OP.mult, OP.mult)
                nc.vector.scalar_tensor_tensor(scr[:, 1:2], s12sb[:, 1:2],
                                               k1, scr[:, 0:1],
                                               OP.mult, OP.add)
                nc.scalar.activation(scr[:, 2:3], scr[:, 1:2], AF.Ln)
                nc.scalar.activation(ab[:, 0:1], scr[:, 2:3], AF.Exp,
                                     scale=-0.5)
                nc.vector.scalar_tensor_tensor(ab[:, 1:2], s12sb[:, 0:1],
                                               -1.0 / NTOT, ab[:, 0:1],
                                               OP.mult, OP.mult)

                # broadcast (inv_std, bias) to 64 partitions via ones matmul
                ps_b = p_pss.tile([64, 2], F32, tag="ps_b")
                nc.tensor.matmul(ps_b[:], onesr_sb[:], ab[:],
                                 start=True, stop=True)
                ab64 = p_s1.tile([64, 2], F32, tag="ab64")
                nc.vector.tensor_copy(ab64[:], ps_b[:])

                # rn = relu(inv_std * t + bias)   (into scr64)
                nc.scalar.activation(scr64[:], t64[:], AF.Relu,
                                     scale=ab64[:, 0:1], bias=ab64[:, 1:2])

                # cT[5, B] = wpT.T @ rn + bp
                c5 = p_s1.tile([5, B], F32, tag="c5")
                for j in range(B // 512):
                    ps_c = p_psc.tile([5, 512], F32, tag="ps_c")
                    nc.tensor.matmul(ps_c[:], wpT_sb[:],
                                     scr64[:, ts(j, 512)],
                                     start=True, stop=True)
                    nc.vector.tensor_scalar(c5[:, ts(j, 512)], ps_c[:],
                                            bp_sb[:, 0:1], None, OP.add)

                # transpose own 4 row blocks to [128, 5]
                for t in range(NPT):
                    ps_t = p_pst.tile([128, 5], F32, tag="ps_t")
                    nc.tensor.transpose(ps_t[:], c5[:, ts(t, 128)],
                                        eye5_sb[:])
                    nc.vector.tensor_copy(ctiles[t][:], ps_t[:])

            # ---- stage 2: dy = poly(eta); all compute on DVE ----
            with (
                tc.tile_pool(name="eta", bufs=2) as p_eta,
                tc.tile_pool(name="u", bufs=2) as p_u,
                tc.tile_pool(name="g", bufs=2) as p_g,
            ):
                W = NDT * CT

                def stt(out, in0, scalar, in1):
                    nc.vector.scalar_tensor_tensor(out, in0, scalar, in1,
                                                   OP.add, OP.mult)

                for _rep in range(repeat):
                    for dt_ in range(NPT // NDT):
                        rows = slice(dt_ * 128 * NDT, (dt_ + 1) * 128 * NDT)
                        eta3 = eta_d[rows, :].rearrange("(n p) c -> p n c",
                                                        p=128)
                        dy3 = dy_d[rows, :].rearrange("(n p) c -> p n c",
                                                      p=128)
                        eta_t = p_eta.tile([128, W], F32, tag="eta")
                        nc.sync.dma_start(
                            eta_t[:].rearrange("p (n c) -> p n c", n=NDT),
                            eta3)
                        u_t = p_u.tile([128, W], F32, tag="u")
                        # u = eta^3 (coefficient-free: full width)
                        nc.vector.tensor_tensor(u_t[:], eta_t[:], eta_t[:],
                                                OP.mult)
                        nc.vector.tensor_tensor(u_t[:], u_t[:], eta_t[:],
                                                OP.mult)
                        g_t = p_g.tile([128, W], F32, tag="g")
                        for n_ in range(NDT):
                            cs = ctiles[NDT * dt_ + n_]
                            c0, c1, c2, c3, c4 = (cs[:, k:k + 1]
                                                  for k in range(5))
                            sl = slice(n_ * CT, (n_ + 1) * CT)
                            # g = c4*u + c3
                            nc.vector.tensor_scalar(g_t[:, sl], u_t[:, sl],
                                                    c4, c3, OP.mult, OP.add)
                            # g = g*u; g = (g+c2)*u; g = (g+c1)*u
                            nc.vector.tensor_tensor(g_t[:, sl], g_t[:, sl],
                                                    u_t[:, sl], OP.mult)
                            stt(g_t[:, sl], g_t[:, sl], c2, u_t[:, sl])
                            stt(g_t[:, sl], g_t[:, sl], c1, u_t[:, sl])
                            # dy = (g + c0)*eta
                            stt(g_t[:, sl], g_t[:, sl], c0, eta_t[:, sl])
                        nc.scalar.dma_start(
                            dy3,
                            g_t[:].rearrange("p (n c) -> p n c", n=NDT))
    nc.compile()
    return nc


def _host_prep(physical_params, W1, b1, W2, b2):
    pp = np.ascontiguousarray(physical_params, dtype=np.float32)
    W1 = np.asarray(W1, dtype=np.float32)
    b1 = np.asarray(b1, dtype=np.float32)
    W2 = np.asarray(W2, dtype=np.float32)
    b2 = np.asarray(b2, dtype=np.float32)

    # fused MLP (no activation between the linears) + fold p_k
    Weff = W2 @ W1                       # [5, 64]
    beff = W2 @ b1 + b2                  # [5]
    Wp = POLY_POWERS[:, None] * Weff     # [5, 64]
    bp = (POLY_POWERS * beff)[:, None]   # [5, 1]

    # replication+scale matrix: rm[i, i*16+j] = PS_POWERS[j]
    rm = np.zeros((4, 64), np.float32)
    for i in range(4):
        rm[i, i * 16:(i + 1) * 16] = PS_POWERS

    consts = {
        "rm": rm,
        "wpT": np.ascontiguousarray(Wp.T),
        "bp": np.ascontiguousarray(bp),
        "ones64": np.ones((64, 1), np.float32),
        "onesr": np.ones((1, 64), np.float32),
        "eye5": np.eye(5, dtype=np.float32),
    }
    return np.ascontiguousarray(pp.T), consts


def kernel(physical_params, eta, W1, b1, W2, b2):
    from concourse.bass_utils import run_bass_kernel_spmd

    eta = np.ascontiguousarray(eta, dtype=np.float32)
    pT, consts = _host_prep(physical_params, W1, b1, W2, b2)

    if "nc" not in _cache:
        _cache["nc"] = _build_nc()
    nc = _cache["nc"]

    in_maps = []
    for g in range(NCORES):
        m = dict(consts)
        m["eta"] = eta[g * RPC:(g + 1) * RPC]
        m["pT"] = np.ascontiguousarray(np.roll(pT, -g * RPC, axis=1))
        in_maps.append(m)

    res = run_bass_kernel_spmd(nc, in_maps, core_ids=list(range(NCORES)))
    _cache["last_results"] = res
    out = np.concatenate([res.results[g]["dy"] for g in range(NCORES)], axis=0)
    return out



# revision 2
# speedup vs baseline: 24.4305x; 24.4305x over previous
"""Trainium2 Bass kernel for nn_CANN_39994735460546.

Reference semantics:
  t    = (physical_params[:, :, None] ** PS_POWERS).reshape(B, 64)
  norm = (t - t.mean()) / t.std(ddof=1)          # global scalar stats
  h    = relu(norm) @ W1.T + b1
  c    = h @ W2.T + b2                            # [B, 5]
  dy[b, j] = sum_k c[b,k] * p_k * eta[b,j]^(p_k - 1),  p = [2,5,8,11,14]

Device strategy (8 NeuronCores, pure data parallel over eta rows; each core
owns 512 rows; stage 1 replicated on every core so no collectives needed):
  - No activation between the two linears -> fuse on host:
      Weff = W2 @ W1, beff = W2 @ b1 + b2, and fold the p_k factors in.
  - Stage 1 (tiny): params.T packed [8, 2048] (rolled per core so its own
    rows come first -> identical SPMD program on all cores; column halves
    stacked on the partition axis so elementwise ops use 128 partitions),
    ln on ACT, replicate+scale to [128, 2048] via a PE matmul with a
    host-built block-diagonal (8 x 128) selection matrix holding
    PS_POWERS, exp on ACT, global stats via DVE accum_out + a ones-matmul
    partition reduction, relu-normalize ONLY the own 512 batch rows,
    one matmul to coefficients cT [5, 512], PE-transpose 128-col slices
    to [128, 5] per-row-block coefficient tiles.
  - Stage 2 (the heavy part): with u = eta^3,
      dy = ((((c4*u + c3)*u + c2)*u + c1)*u + c0)*eta
    split across three engines per 128-row pass:
      ACT : ln(eta), u = exp(3*ln)           (2 ops)
      DVE : ts (c4*u+c3) + 3 stt Horner steps (4 ops; ts runs in the
            fp32 2x perf mode, stt ~1 elem/lane/cycle)
      Pool: the one coefficient-free multiply g *= u (tensor_tensor;
            Pool cannot run AP-scalar ops, but tt it can)
    Measured per-[128,4096]-pass cost ~15us DVE / ~14us ACT / ~10us Pool,
    pipelined across 4 passes with ping-pong tiles.
  - DMA: loads on the SP HWDGE ring, stores on the ACT HWDGE ring
    ([128, 4096] f32 tiles = 16KB contiguous per partition).
"""

import sys
import numpy as np

sys.path.insert(0, "/opt/trn_rl_repo")

B = 4096
L = 4096
NCORES = 8
RPC = B // NCORES          # rows per core = 512
NPT = RPC // 128           # 128-row passes per core = 4
CT = 4096                  # row width
NTOT = float(B * 64)       # elements in t for the global stats
HB = B // 2                # packed stage-1 width = 2048

PS_POWERS = np.array([-5.0, -4.0, -3.0, -2.0, -1.5, -1.0, -0.5, 0.0,
                      0.5, 2.0, 1.0 / 3.0, 3.0, 0.25, 4.0, 0.2, 5.0],
                     dtype=np.float32)
POLY_POWERS = np.array([2.0, 5.0, 8.0, 11.0, 14.0], dtype=np.float32)

USE_POOL = True

_cache = {}


def _build_nc(loop_n=None):
    """loop_n=None -> single-shot kernel (the graded path).
    loop_n=N -> stage 2 wrapped in a For_i(0, N) hardware loop, used by
    test.py for repeat-amplified timing at constant NEFF size."""
    import concourse.bass as bass
    import concourse.tile as tile
    from concourse import bacc, mybir

    F32 = mybir.dt.float32
    AF = mybir.ActivationFunctionType
    OP = mybir.AluOpType
    ts = bass.ts

    k1 = 1.0 / (NTOT - 1.0)
    k2 = 1.0 / (NTOT * (NTOT - 1.0))

    nc = bacc.Bacc("TRN2", target_bir_lowering=False, debug=False,
                   num_devices=NCORES)

    eta_d = nc.dram_tensor("eta", [RPC, L], F32, kind="ExternalInput").ap()
    pT8_d = nc.dram_tensor("pT8", [8, HB], F32, kind="ExternalInput").ap()
    rm2_d = nc.dram_tensor("rm2", [8, 128], F32, kind="ExternalInput").ap()
    wpT_d = nc.dram_tensor("wpT", [64, 5], F32, kind="ExternalInput").ap()
    bp_d = nc.dram_tensor("bp", [5, 1], F32, kind="ExternalInput").ap()
    ones128_d = nc.dram_tensor("ones128", [128, 1], F32,
                               kind="ExternalInput").ap()
    onesr_d = nc.dram_tensor("onesr", [1, 128], F32, kind="ExternalInput").ap()
    eye5_d = nc.dram_tensor("eye5", [5, 5], F32, kind="ExternalInput").ap()
    dy_d = nc.dram_tensor("dy", [RPC, L], F32, kind="ExternalOutput").ap()

    with tile.TileContext(nc) as tc:
        with (
            tc.tile_pool(name="consts", bufs=1) as p_const,
            tc.tile_pool(name="ps_small", bufs=1, space="PSUM") as p_pss,
            tc.tile_pool(name="ps_r", bufs=2, space="PSUM") as p_psr,
            tc.tile_pool(name="ps_t", bufs=2, space="PSUM") as p_pst,
        ):
            # ---- constants (on the store ring; loads keep the SP ring) --
            rm2_sb = p_const.tile([8, 128], F32, tag="rm2")
            nc.scalar.dma_start(rm2_sb[:], rm2_d)
            wpT_sb = p_const.tile([64, 5], F32, tag="wpT")
            nc.scalar.dma_start(wpT_sb[:], wpT_d)
            bp_sb = p_const.tile([5, 1], F32, tag="bp")
            nc.scalar.dma_start(bp_sb[:], bp_d)
            ones128_sb = p_const.tile([128, 1], F32, tag="ones128")
            nc.scalar.dma_start(ones128_sb[:], ones128_d)
            onesr_sb = p_const.tile([1, 128], F32, tag="onesr")
            nc.scalar.dma_start(onesr_sb[:], onesr_d)
            eye5_sb = p_const.tile([5, 5], F32, tag="eye5")
            nc.scalar.dma_start(eye5_sb[:], eye5_d)
            ctiles = [p_const.tile([128, 5], F32, tag=f"ct{t}",
                                   name=f"ct{t}") for t in range(NPT)]

            # ---- stage 1 in its own (stack-freed) scratch pool ----
            with tc.tile_pool(name="s1", bufs=1) as p_s1:
                pT_sb = p_s1.tile([8, HB], F32, tag="pT")
                nc.sync.dma_start(pT_sb[:], pT8_d)
                t128 = p_s1.tile([128, HB], F32, tag="t128")
                scr128 = p_s1.tile([128, HB], F32, tag="scr128")
                s12 = p_s1.tile([128, 2], F32, tag="s12")

                # ln(params); t128 = exp(rm2.T @ ln)  (rm2 replicates 8->128
                # rows and scales by PS_POWERS)
                nc.scalar.activation(pT_sb[:], pT_sb[:], AF.Ln)
                for j in range(HB // 512):
                    ps_r = p_psr.tile([128, 512], F32, tag="ps_r")
                    nc.tensor.matmul(ps_r[:], rm2_sb[:], pT_sb[:, ts(j, 512)],
                                     start=True, stop=True)
                    nc.scalar.activation(t128[:, ts(j, 512)], ps_r[:], AF.Exp)

                # S2 = sum(t^2), S1 = sum(t) per partition (DVE accum_out)
                nc.vector.scalar_tensor_tensor(scr128[:], t128[:], 1.0,
                                               t128[:], OP.mult, OP.mult,
                                               accum_out=s12[:, 1:2])
                nc.vector.tensor_scalar(scr128[:], t128[:], 1.0, 0.0, OP.mult,
                                        OP.add, accum_out=s12[:, 0:1])

                # cross-partition: [1,2] = ones128.T @ s12
                ps_s = p_pss.tile([1, 2], F32, tag="ps_s")
                nc.tensor.matmul(ps_s[:], ones128_sb[:], s12[:],
                                 start=True, stop=True)
                s12sb = p_s1.tile([1, 2], F32, tag="s12sb")
                nc.vector.tensor_copy(s12sb[:], ps_s[:])

                # var = S2/(N-1) - S1^2/(N(N-1)); inv_std = exp(-0.5 ln var)
                scr = p_s1.tile([1, 4], F32, tag="scr")
                ab = p_s1.tile([1, 2], F32, tag="ab")
                nc.vector.tensor_scalar(scr[:, 0:1], s12sb[:, 0:1],
                                        s12sb[:, 0:1], -k2, OP.mult, OP.mult)
                nc.vector.scalar_tensor_tensor(scr[:, 1:2], s12sb[:, 1:2],
                                               k1, scr[:, 0:1],
                                               OP.mult, OP.add)
                nc.scalar.activation(scr[:, 2:3], scr[:, 1:2], AF.Ln)
                nc.scalar.activation(ab[:, 0:1], scr[:, 2:3], AF.Exp,
                                     scale=-0.5)
                nc.vector.scalar_tensor_tensor(ab[:, 1:2], s12sb[:, 0:1],
                                               -1.0 / NTOT, ab[:, 0:1],
                                               OP.mult, OP.mult)

                # broadcast (inv_std, bias) to 128 partitions via ones matmul
                ps_b = p_pss.tile([128, 2], F32, tag="ps_b")
                nc.tensor.matmul(ps_b[:], onesr_sb[:], ab[:],
                                 start=True, stop=True)
                ab128 = p_s1.tile([128, 2], F32, tag="ab128")
                nc.vector.tensor_copy(ab128[:], ps_b[:])

                # rn = relu(inv_std * t + bias), own 512 batch rows only
                # (they sit in columns 0:512 of the top partition half)
                rn = p_s1.tile([64, 512], F32, tag="rn")
                nc.scalar.activation(rn[:], t128[0:64, 0:512], AF.Relu,
                                     scale=ab128[0:64, 0:1],
                                     bias=ab128[0:64, 1:2])

                # cT[5, 512] = wpT.T @ rn + bp
                ps_c = p_pss.tile([5, 512], F32, tag="ps_c")
                nc.tensor.matmul(ps_c[:], wpT_sb[:], rn[:],
                                 start=True, stop=True)
                c5 = p_s1.tile([5, 512], F32, tag="c5")
                nc.vector.tensor_scalar(c5[:], ps_c[:], bp_sb[:, 0:1], None,
                                        OP.add)

                # transpose own 4 row blocks to [128, 5]
                for t in range(NPT):
                    ps_t = p_pst.tile([128, 5], F32, tag="ps_t")
                    nc.tensor.transpose(ps_t[:], c5[:, ts(t, 128)],
                                        eye5_sb[:])
                    nc.vector.tensor_copy(ctiles[t][:], ps_t[:])

            # ---- stage 2: dy = poly(eta), ACT + DVE + Pool pipelined ----
            with tc.tile_pool(name="s2", bufs=1) as p_s2:
                eta_sl = [p_s2.tile([128, CT], F32, tag=f"eta{s}",
                                    name=f"eta{s}") for s in range(2)]
                u_sl = [p_s2.tile([128, CT], F32, tag=f"u{s}",
                                  name=f"u{s}") for s in range(2)]
                g_sl = [p_s2.tile([128, CT], F32, tag=f"g{s}",
                                  name=f"g{s}") for s in range(2)]

                def stage2_body():
                    for i in range(NPT):
                        s = i % 2
                        eta_t, u_t, g_t = eta_sl[s], u_sl[s], g_sl[s]
                        cs = ctiles[i]
                        c0, c1, c2, c3, c4 = (cs[:, k:k + 1]
                                              for k in range(5))
                        nc.sync.dma_start(eta_t[:], eta_d[ts(i, 128), :])
                        # u = eta^3 = exp(3 ln eta) on ACT
                        nc.scalar.activation(u_t[:], eta_t[:], AF.Ln)
                        nc.scalar.activation(u_t[:], u_t[:], AF.Exp,
                                             scale=3.0)
                        # g = c4*u + c3   (fp32 2x tensor_scalar)
                        nc.vector.tensor_scalar(g_t[:], u_t[:], c4, c3,
                                                OP.mult, OP.add)
                        # g = g*u  (coefficient-free -> Pool's tt)
                        if USE_POOL:
                            nc.gpsimd.tensor_tensor(g_t[:], g_t[:], u_t[:],
                                                    OP.mult)
                        else:
                            nc.vector.scalar_tensor_tensor(
                                g_t[:], g_t[:], 0.0, u_t[:], OP.add, OP.mult)
                        # g = (g+c2)*u; g = (g+c1)*u; g = (g+c0)*eta
                        nc.vector.scalar_tensor_tensor(
                            g_t[:], g_t[:], c2, u_t[:], OP.add, OP.mult)
                        nc.vector.scalar_tensor_tensor(
                            g_t[:], g_t[:], c1, u_t[:], OP.add, OP.mult)
                        nc.vector.scalar_tensor_tensor(
                            g_t[:], g_t[:], c0, eta_t[:], OP.add, OP.mult)
                        nc.scalar.dma_start(dy_d[ts(i, 128), :], g_t[:])

                if loop_n is None:
                    stage2_body()
                else:
                    with tc.For_i(0, loop_n, 1):
                        stage2_body()
    nc.compile()
    return nc


def _host_prep(physical_params, W1, b1, W2, b2):
    pp = np.ascontiguousarray(physical_params, dtype=np.float32)
    W1 = np.asarray(W1, dtype=np.float32)
    b1 = np.asarray(b1, dtype=np.float32)
    W2 = np.asarray(W2, dtype=np.float32)
    b2 = np.asarray(b2, dtype=np.float32)

    # fused MLP (no activation between the linears) + fold p_k
    Weff = W2 @ W1                       # [5, 64]
    beff = W2 @ b1 + b2                  # [5]
    Wp = POLY_POWERS[:, None] * Weff     # [5, 64]
    bp = (POLY_POWERS * beff)[:, None]   # [5, 1]

    # block-diag replication+scale matrix:
    # rm2[i, i*16+j] = PS_POWERS[j] for i<4, shifted to partitions 64+ for
    # the second column half
    rm2 = np.zeros((8, 128), np.float32)
    for i in range(4):
        rm2[i, i * 16:(i + 1) * 16] = PS_POWERS
        rm2[4 + i, 64 + i * 16:64 + (i + 1) * 16] = PS_POWERS

    consts = {
        "rm2": rm2,
        "wpT": np.ascontiguousarray(Wp.T),
        "bp": np.ascontiguousarray(bp),
        "ones128": np.ones((128, 1), np.float32),
        "onesr": np.ones((1, 128), np.float32),
        "eye5": np.eye(5, dtype=np.float32),
    }
    return np.ascontiguousarray(pp.T), consts


def _pack_pT(pT_rolled):
    """[4, B] -> [8, B/2]: column halves stacked on the partition axis."""
    return np.ascontiguousarray(
        np.concatenate([pT_rolled[:, :HB], pT_rolled[:, HB:]], axis=0))


def _in_maps(eta, pT, consts):
    in_maps = []
    for g in range(NCORES):
        m = dict(consts)
        m["eta"] = eta[g * RPC:(g + 1) * RPC]
        m["pT8"] = _pack_pT(np.roll(pT, -g * RPC, axis=1))
        in_maps.append(m)
    return in_maps


def kernel(physical_params, eta, W1, b1, W2, b2):
    from concourse.bass_utils import run_bass_kernel_spmd

    eta = np.ascontiguousarray(eta, dtype=np.float32)
    pT, consts = _host_prep(physical_params, W1, b1, W2, b2)

    if "nc" not in _cache:
        _cache["nc"] = _build_nc()
    nc = _cache["nc"]

    res = run_bass_kernel_spmd(nc, _in_maps(eta, pT, consts),
                               core_ids=list(range(NCORES)))
    _cache["last_results"] = res
    out = np.concatenate([res.results[g]["dy"] for g in range(NCORES)], axis=0)
    return out
